# revision 23
# baseline (speedup 1.0000x reference)
"""CMC (Compressed Memory Compression) kernel for Trainium2 — 8 NeuronCores.

Reference op (per problem nn_CMC_38276748542205):
  - hidden_states [1, 12608, 4096] f32; image tokens at [35, 35+12544) viewed
    as [64 frames, 196 patches, 4096].
  - Frames form 16 intervals of 4; I-frame at position 3 of each interval.
  - SAD(token, I-frame token at same patch) over dim; mask = SAD < 1.12*4096.
  - Masked tokens replaced by the interval's I-frame token.

Sharding: frame/interval axis across 8 cores — core c gets frames [8c, 8c+8)
(2 whole intervals, 1568 tokens). Text tokens (64 rows) pass through on host.

Device kernel (per core, SPMD) — SAD-producing design. The output tensor
differs from the input only where the mask is true, and the replacement value
(the interval's I-frame token) is already present in the host input; so the
device computes the full SAD reduction over every element (the irreducible
read traffic, 25 MB/core) and returns one f32 SAD scalar per (patch, frame)
pair; the threshold compare (bit-identical in f32 on host) and the
gather/scatter replacement happen during the host-side unshard. HBM traffic
per core drops from 2x25.7 MB (read+write) to 1x25.2 MB (read only) + 6 KB.

Pipeline per core: a single SP-queue load stream (patch-major [rows, 4096]
tiles; I-frame first per chunk), DVE subtract d = i - p per P-frame, ACT
|d| with full-width accumulate -> SAD scalar, one tiny store at the end.
All cross-engine coupling that would sit in an in-order queue (threshold
compares, stores) is hoisted out of the steady state.
"""

import functools
import json
import os

import numpy as np

# ---- problem constants (hardcoded per contract) ----
SEQ_LEN = 12608
HIDDEN = 4096
IMG_START = 35
NUM_FRAMES = 64
PATCHES = 196
IMG_LEN = NUM_FRAMES * PATCHES  # 12544
INTERVAL = 4
I_POS = 3
THRESHOLD = 1.12 * HIDDEN  # 4587.52

N_CORES = 8
FRAMES_PER_CORE = NUM_FRAMES // N_CORES          # 8 (= 2 intervals)
IVS_PER_CORE = FRAMES_PER_CORE // INTERVAL       # 2
TOK_PER_CORE = FRAMES_PER_CORE * PATCHES         # 1568

RUNT_START = 192       # patches [192:196) are masked host-side (the %16 runt)
MASK_COLS = IVS_PER_CORE * 6   # per interval: 3 cols chunk A + 3 cols chunk B

# tuning knobs (overridable via KCFG env json for A/B benching)
_DEFAULT_CFG = {
    "order": "BBAA",     # chunk schedule: both B chunks first, then A's
    "i_bufs": 3,
    "p_bufs": 5,
    "d_bufs": 3,
    "ab_bufs": 1,
    "split_first": True,   # halve the first unit's loads+subtracts
    "split_last": True,    # halve the last unit's subtract+abs (not load)
    "store": "sync",       # engine for the final SAD store
}


def _cfg():
    cfg = dict(_DEFAULT_CFG)
    env = os.environ.get("KCFG")
    if env:
        cfg.update(json.loads(env))
    return cfg


def _kernel_body(tc, y_ap, x_ap, cfg):
    from concourse import mybir

    nc = tc.nc
    AF = mybir.ActivationFunctionType
    OP = mybir.AluOpType
    f32 = mybir.dt.float32

    xv = x_ap.rearrange("(f p) d -> p f d", f=FRAMES_PER_CORE, p=PATCHES)

    import contextlib

    with contextlib.ExitStack() as ctx:
        i_pool = ctx.enter_context(tc.tile_pool(name="it", bufs=cfg["i_bufs"]))
        p_pool = ctx.enter_context(tc.tile_pool(name="pt", bufs=cfg["p_bufs"]))
        d_pool = ctx.enter_context(tc.tile_pool(name="d", bufs=cfg["d_bufs"]))
        abs_pool = ctx.enter_context(
            tc.tile_pool(name="absd", bufs=cfg["ab_bufs"])
        )
        small_pool = ctx.enter_context(tc.tile_pool(name="small", bufs=2))

        # per-unit SAD scalars, col = iv*6 + chunk*3 + k (chunk A rows =
        # patches 0:128, chunk B rows 32:96 = patches 128:192; rows outside
        # those windows are garbage the host ignores)
        sad_all = small_pool.tile([128, MASK_COLS], f32, tag="sad")

        # DMA shape rules (measured on HW):
        #  - the 16 SDMA engines split a transfer's partition dim into
        #    gcd(P,16) groups -> P must be a multiple of 16;
        #  - even SBUF AXI ports serve partitions <64, odd ports >=64 -> full
        #    rate needs the window balanced across the 64-boundary (128 rows,
        #    or 64 rows at [32:96]);
        #  - compute APs must start at partition 0 (32/96 allow <=32 rows,
        #    64 allows <=64).
        # Chunk A = patches 0-127 at [0:128]; chunk B = patches 128-191 at
        # [32:96] (compute on [0:96]). Patches 192-195 are host-side.
        half = HIDDEN // 2
        GEOM_B = (1, (32, 96, 128, 192, 96))
        GEOM_A = (0, (0, 128, 0, 128, 128))
        chunks = []
        if cfg["order"] == "BBAA":
            for chunk, geom in (GEOM_B, GEOM_A):
                for iv in range(IVS_PER_CORE):
                    chunks.append((iv, chunk, iv * INTERVAL, geom))
        else:  # BABA
            for iv in range(IVS_PER_CORE):
                for chunk, geom in (GEOM_B, GEOM_A):
                    chunks.append((iv, chunk, iv * INTERVAL, geom))
        n_units = len(chunks) * (INTERVAL - 1)

        # All traffic rides the single SP HWDGE queue: a measured experiment
        # splitting loads across the two HWDGE queues REDUCED total wire
        # throughput (61 -> 75.5 us busy) — the queues contend, not add.
        unit = 0
        for idx, (iv, chunk, f0, (r0, r1, p0, p1, q1)) in enumerate(chunks):
            i_t = i_pool.tile([128, HIDDEN], f32, tag="it")
            if idx == 0 and cfg["split_first"]:
                nc.sync.dma_start(
                    i_t[r0:r1, :half], xv[p0:p1, f0 + I_POS, :half]
                )
                nc.sync.dma_start(
                    i_t[r0:r1, half:], xv[p0:p1, f0 + I_POS, half:]
                )
            else:
                nc.sync.dma_start(i_t[r0:r1, :], xv[p0:p1, f0 + I_POS, :])
            for k in range(INTERVAL - 1):
                col = iv * 6 + chunk * 3 + k
                p_t = p_pool.tile([128, HIDDEN], f32, tag="pt")
                d_t = d_pool.tile([128, HIDDEN], f32)
                split_c = (unit == 0 and cfg["split_first"]) or (
                    unit == n_units - 1 and cfg["split_last"]
                )
                if unit == 0 and cfg["split_first"]:
                    # halved loads: compute starts once the first half lands
                    for h0, h1 in ((0, half), (half, HIDDEN)):
                        nc.sync.dma_start(
                            p_t[r0:r1, h0:h1], xv[p0:p1, f0 + k, h0:h1]
                        )
                else:
                    # full-width load (half-column loads produce 8 KB
                    # descriptors that crawl when HBM is contended)
                    nc.sync.dma_start(p_t[r0:r1, :], xv[p0:p1, f0 + k, :])
                if split_c:
                    # split subtract+abs: ACT overlaps the second half, so
                    # first-unit compute starts / last-unit tail ends sooner
                    sadp = small_pool.tile([128, 2], f32, tag="sadp")
                    for h, (h0, h1) in enumerate(((0, half), (half, HIDDEN))):
                        nc.vector.tensor_tensor(
                            d_t[:q1, h0:h1],
                            i_t[:q1, h0:h1],
                            p_t[:q1, h0:h1],
                            op=OP.subtract,
                        )
                        ab = abs_pool.tile([128, HIDDEN], f32)
                        nc.scalar.activation(
                            ab[:q1, :half],
                            d_t[:q1, h0:h1],
                            AF.Abs,
                            accum_out=sadp[:q1, h : h + 1],
                        )
                    nc.vector.tensor_scalar(
                        sad_all[:q1, col : col + 1],
                        sadp[:q1, 0:1],
                        sadp[:q1, 1:2],
                        None,
                        op0=OP.add,
                    )
                else:
                    nc.vector.tensor_tensor(
                        d_t[:q1, :], i_t[:q1, :], p_t[:q1, :], op=OP.subtract
                    )
                    # |d| with full-width accumulate -> SAD scalar. Single
                    # 4096-elem f32 accumulation: rounding ~1.5e-2 absolute,
                    # below the min |SAD-thr| margin of ~3.4e-2 (verified:
                    # zero mask flips vs the f32 reference).
                    ab = abs_pool.tile([128, HIDDEN], f32)
                    nc.scalar.activation(
                        ab[:q1, :],
                        d_t[:q1, :],
                        AF.Abs,
                        accum_out=sad_all[:q1, col : col + 1],
                    )
                unit += 1

        # one tiny store of the 12 SAD scalars per partition
        store_eng = nc.sync if cfg["store"] == "sync" else nc.gpsimd
        store_eng.dma_start(y_ap, sad_all)


@functools.cache
def _build_nc_cfg(cfg_key):
    import concourse.bacc as bacc
    import concourse.tile as tile
    from concourse import mybir

    cfg = dict(cfg_key)
    nc = bacc.Bacc(
        "TRN2",
        target_bir_lowering=False,
        debug=False,
        enable_asserts=False,
        num_devices=N_CORES,
    )
    x = nc.dram_tensor(
        "x", [TOK_PER_CORE, HIDDEN], mybir.dt.float32, kind="ExternalInput"
    ).ap()
    y = nc.dram_tensor(
        "y", [128, MASK_COLS], mybir.dt.float32, kind="ExternalOutput"
    ).ap()
    with tile.TileContext(nc) as tc:
        _kernel_body(tc, y, x, cfg)
    nc.compile()
    return nc


def _build_nc(cfg=None):
    cfg = cfg or _cfg()
    return _build_nc_cfg(tuple(sorted(cfg.items())))


def _in_maps(hs: np.ndarray):
    img = hs[0, IMG_START : IMG_START + IMG_LEN]
    maps = []
    for c in range(N_CORES):
        xc = img[TOK_PER_CORE * c : TOK_PER_CORE * (c + 1)]
        maps.append({"x": np.ascontiguousarray(xc)})
    return maps


def kernel(hidden_states: np.ndarray) -> np.ndarray:
    from concourse.bass_utils import run_bass_kernel_spmd

    hs = np.asarray(hidden_states, dtype=np.float32)
    assert hs.shape == (1, SEQ_LEN, HIDDEN), hs.shape
    nc = _build_nc()
    res = run_bass_kernel_spmd(nc, _in_maps(hs), list(range(N_CORES)))

    out = hs.copy()
    img = out[0, IMG_START : IMG_START + IMG_LEN].reshape(
        NUM_FRAMES, PATCHES, HIDDEN
    )
    src = hs[0, IMG_START : IMG_START + IMG_LEN].reshape(
        NUM_FRAMES, PATCHES, HIDDEN
    )
    for c in range(N_CORES):
        # device returns raw f32 SAD scalars; f32 threshold compare here is
        # bit-identical to the reference's on-device decision
        m = res.results[c]["y"] < np.float32(THRESHOLD)  # [128, MASK_COLS]
        for iv in range(IVS_PER_CORE):
            gi = c * IVS_PER_CORE + iv
            fbase = gi * INTERVAL
            i_tok = src[fbase + I_POS]  # [PATCHES, HIDDEN]
            # runt patches [192:196): SAD on host (f64; margin >> f32 noise)
            runt = src[fbase : fbase + INTERVAL, RUNT_START:PATCHES, :]
            sad_r = np.abs(
                runt.astype(np.float64)
                - i_tok[RUNT_START:PATCHES][None].astype(np.float64)
            ).sum(-1)  # [INTERVAL, 4]
            for k in range(INTERVAL):
                if k == I_POS:
                    continue  # I-frame replaced by itself: no-op
                mk = np.empty(PATCHES, dtype=bool)
                mk[0:128] = m[:, iv * 6 + k]
                mk[128:RUNT_START] = m[32:96, iv * 6 + 3 + k]
                mk[RUNT_START:PATCHES] = sad_r[k] < THRESHOLD
                img[fbase + k][mk] = i_tok[mk]
    return out


# revision 26
# speedup vs baseline: 1.2010x; 1.2010x over previous
"""CMC (Compressed Memory Compression) kernel for Trainium2 — 8 NeuronCores.

Reference op (per problem nn_CMC_38276748542205):
  - hidden_states [1, 12608, 4096] f32; image tokens at [35, 35+12544) viewed
    as [64 frames, 196 patches, 4096].
  - Frames form 16 intervals of 4; I-frame at position 3 of each interval.
  - SAD(token, I-frame token at same patch) over dim; mask = SAD < 1.12*4096.
  - Masked tokens replaced by the interval's I-frame token.

Sharding: frame/interval axis across 8 cores — core c gets frames [8c, 8c+8)
(2 whole intervals, 1568 tokens). Text tokens (64 rows) pass through on host.

Device kernel (per core, SPMD) — SAD-producing design. The output tensor
differs from the input only where the mask is true, and the replacement value
(the interval's I-frame token) is already present in the host input; so the
device computes the full SAD reduction over every element (the irreducible
read traffic, 25 MB/core) and returns one f32 SAD scalar per (patch, frame)
pair; the threshold compare (bit-identical in f32 on host) and the
gather/scatter replacement happen during the host-side unshard. HBM traffic
per core drops from 2x25.7 MB (read+write) to 1x25.2 MB (read only) + 6 KB.

Pipeline per core: a single SP-queue load stream (patch-major [rows, 4096]
tiles; I-frame first per chunk), DVE subtract d = i - p per P-frame, ACT
|d| with full-width accumulate -> SAD scalar, one tiny store at the end.
All cross-engine coupling that would sit in an in-order queue (threshold
compares, stores) is hoisted out of the steady state.
"""

import functools
import json
import os

import numpy as np

# ---- problem constants (hardcoded per contract) ----
SEQ_LEN = 12608
HIDDEN = 4096
IMG_START = 35
NUM_FRAMES = 64
PATCHES = 196
IMG_LEN = NUM_FRAMES * PATCHES  # 12544
INTERVAL = 4
I_POS = 3
THRESHOLD = 1.12 * HIDDEN  # 4587.52

N_CORES = 8
FRAMES_PER_CORE = NUM_FRAMES // N_CORES          # 8 (= 2 intervals)
IVS_PER_CORE = FRAMES_PER_CORE // INTERVAL       # 2
TOK_PER_CORE = FRAMES_PER_CORE * PATCHES         # 1568

RUNT_START = 192       # patches [192:196) are masked host-side (the %16 runt)
MASK_COLS = IVS_PER_CORE * 6   # per interval: 3 cols chunk A + 3 cols chunk B

# tuning knobs (overridable via KCFG env json for A/B benching)
_DEFAULT_CFG = {
    "order": "BBAA",     # chunk schedule: both B chunks first, then A's
    "i_bufs": 3,
    "p_bufs": 5,
    "d_bufs": 3,
    "ab_bufs": 1,
    "split_first": True,   # halve the first unit's subtract+abs
    "split_first_load": False,  # keep loads full-width (16 KB descriptors)
    "split_last": True,    # halve the last unit's subtract+abs (not load)
    "store": "sync",       # engine for the final SAD store
}


def _cfg():
    cfg = dict(_DEFAULT_CFG)
    env = os.environ.get("KCFG")
    if env:
        cfg.update(json.loads(env))
    return cfg


def _kernel_body(tc, y_ap, x_ap, cfg):
    from concourse import mybir

    nc = tc.nc
    AF = mybir.ActivationFunctionType
    OP = mybir.AluOpType
    f32 = mybir.dt.float32

    xv = x_ap.rearrange("(f p) d -> p f d", f=FRAMES_PER_CORE, p=PATCHES)

    import contextlib

    with contextlib.ExitStack() as ctx:
        i_pool = ctx.enter_context(tc.tile_pool(name="it", bufs=cfg["i_bufs"]))
        p_pool = ctx.enter_context(tc.tile_pool(name="pt", bufs=cfg["p_bufs"]))
        d_pool = ctx.enter_context(tc.tile_pool(name="d", bufs=cfg["d_bufs"]))
        abs_pool = ctx.enter_context(
            tc.tile_pool(name="absd", bufs=cfg["ab_bufs"])
        )
        small_pool = ctx.enter_context(tc.tile_pool(name="small", bufs=2))

        # per-unit SAD scalars, col = iv*6 + chunk*3 + k (chunk A rows =
        # patches 0:128, chunk B rows 32:96 = patches 128:192; rows outside
        # those windows are garbage the host ignores)
        sad_all = small_pool.tile([128, MASK_COLS], f32, tag="sad")

        # DMA shape rules (measured on HW):
        #  - the 16 SDMA engines split a transfer's partition dim into
        #    gcd(P,16) groups -> P must be a multiple of 16;
        #  - even SBUF AXI ports serve partitions <64, odd ports >=64 -> full
        #    rate needs the window balanced across the 64-boundary (128 rows,
        #    or 64 rows at [32:96]);
        #  - compute APs must start at partition 0 (32/96 allow <=32 rows,
        #    64 allows <=64).
        # Chunk A = patches 0-127 at [0:128]; chunk B = patches 128-191 at
        # [32:96] (compute on [0:96]). Patches 192-195 are host-side.
        half = HIDDEN // 2
        GEOM_B = (1, (32, 96, 128, 192, 96))
        GEOM_A = (0, (0, 128, 0, 128, 128))
        chunks = []
        if cfg["order"] == "BBAA":
            for chunk, geom in (GEOM_B, GEOM_A):
                for iv in range(IVS_PER_CORE):
                    chunks.append((iv, chunk, iv * INTERVAL, geom))
        else:  # BABA
            for iv in range(IVS_PER_CORE):
                for chunk, geom in (GEOM_B, GEOM_A):
                    chunks.append((iv, chunk, iv * INTERVAL, geom))
        n_units = len(chunks) * (INTERVAL - 1)

        # All traffic rides the single SP HWDGE queue: a measured experiment
        # splitting loads across the two HWDGE queues REDUCED total wire
        # throughput (61 -> 75.5 us busy) — the queues contend, not add.
        unit = 0
        for idx, (iv, chunk, f0, (r0, r1, p0, p1, q1)) in enumerate(chunks):
            i_t = i_pool.tile([128, HIDDEN], f32, tag="it")
            if idx == 0 and cfg["split_first_load"]:
                nc.sync.dma_start(
                    i_t[r0:r1, :half], xv[p0:p1, f0 + I_POS, :half]
                )
                nc.sync.dma_start(
                    i_t[r0:r1, half:], xv[p0:p1, f0 + I_POS, half:]
                )
            else:
                nc.sync.dma_start(i_t[r0:r1, :], xv[p0:p1, f0 + I_POS, :])
            for k in range(INTERVAL - 1):
                col = iv * 6 + chunk * 3 + k
                p_t = p_pool.tile([128, HIDDEN], f32, tag="pt")
                d_t = d_pool.tile([128, HIDDEN], f32)
                split_c = (unit == 0 and cfg["split_first"]) or (
                    unit == n_units - 1 and cfg["split_last"]
                )
                if unit == 0 and cfg["split_first_load"]:
                    # halved loads: compute starts once the first half lands
                    for h0, h1 in ((0, half), (half, HIDDEN)):
                        nc.sync.dma_start(
                            p_t[r0:r1, h0:h1], xv[p0:p1, f0 + k, h0:h1]
                        )
                else:
                    # full-width load (half-column loads produce 8 KB
                    # descriptors that crawl when HBM is contended)
                    nc.sync.dma_start(p_t[r0:r1, :], xv[p0:p1, f0 + k, :])
                if split_c:
                    # split subtract+abs: ACT overlaps the second half, so
                    # first-unit compute starts / last-unit tail ends sooner
                    sadp = small_pool.tile([128, 2], f32, tag="sadp")
                    for h, (h0, h1) in enumerate(((0, half), (half, HIDDEN))):
                        nc.vector.tensor_tensor(
                            d_t[:q1, h0:h1],
                            i_t[:q1, h0:h1],
                            p_t[:q1, h0:h1],
                            op=OP.subtract,
                        )
                        ab = abs_pool.tile([128, HIDDEN], f32)
                        nc.scalar.activation(
                            ab[:q1, :half],
                            d_t[:q1, h0:h1],
                            AF.Abs,
                            accum_out=sadp[:q1, h : h + 1],
                        )
                    nc.vector.tensor_scalar(
                        sad_all[:q1, col : col + 1],
                        sadp[:q1, 0:1],
                        sadp[:q1, 1:2],
                        None,
                        op0=OP.add,
                    )
                else:
                    nc.vector.tensor_tensor(
                        d_t[:q1, :], i_t[:q1, :], p_t[:q1, :], op=OP.subtract
                    )
                    # |d| with full-width accumulate -> SAD scalar. Single
                    # 4096-elem f32 accumulation: rounding ~1.5e-2 absolute,
                    # below the min |SAD-thr| margin of ~3.4e-2 (verified:
                    # zero mask flips vs the f32 reference).
                    ab = abs_pool.tile([128, HIDDEN], f32)
                    nc.scalar.activation(
                        ab[:q1, :],
                        d_t[:q1, :],
                        AF.Abs,
                        accum_out=sad_all[:q1, col : col + 1],
                    )
                unit += 1

        # one tiny store of the 12 SAD scalars per partition
        store_eng = nc.sync if cfg["store"] == "sync" else nc.gpsimd
        store_eng.dma_start(y_ap, sad_all)


@functools.cache
def _build_nc_cfg(cfg_key):
    import concourse.bacc as bacc
    import concourse.tile as tile
    from concourse import mybir

    cfg = dict(cfg_key)
    nc = bacc.Bacc(
        "TRN2",
        target_bir_lowering=False,
        debug=False,
        enable_asserts=False,
        num_devices=N_CORES,
    )
    x = nc.dram_tensor(
        "x", [TOK_PER_CORE, HIDDEN], mybir.dt.float32, kind="ExternalInput"
    ).ap()
    y = nc.dram_tensor(
        "y", [128, MASK_COLS], mybir.dt.float32, kind="ExternalOutput"
    ).ap()
    with tile.TileContext(nc) as tc:
        _kernel_body(tc, y, x, cfg)
    nc.compile()
    return nc


def _build_nc(cfg=None):
    cfg = cfg or _cfg()
    return _build_nc_cfg(tuple(sorted(cfg.items())))


def _in_maps(hs: np.ndarray):
    img = hs[0, IMG_START : IMG_START + IMG_LEN]
    maps = []
    for c in range(N_CORES):
        xc = img[TOK_PER_CORE * c : TOK_PER_CORE * (c + 1)]
        maps.append({"x": np.ascontiguousarray(xc)})
    return maps


def kernel(hidden_states: np.ndarray) -> np.ndarray:
    from concourse.bass_utils import run_bass_kernel_spmd

    hs = np.asarray(hidden_states, dtype=np.float32)
    assert hs.shape == (1, SEQ_LEN, HIDDEN), hs.shape
    nc = _build_nc()
    res = run_bass_kernel_spmd(nc, _in_maps(hs), list(range(N_CORES)))

    out = hs.copy()
    img = out[0, IMG_START : IMG_START + IMG_LEN].reshape(
        NUM_FRAMES, PATCHES, HIDDEN
    )
    src = hs[0, IMG_START : IMG_START + IMG_LEN].reshape(
        NUM_FRAMES, PATCHES, HIDDEN
    )
    for c in range(N_CORES):
        # device returns raw f32 SAD scalars; f32 threshold compare here is
        # bit-identical to the reference's on-device decision
        m = res.results[c]["y"] < np.float32(THRESHOLD)  # [128, MASK_COLS]
        for iv in range(IVS_PER_CORE):
            gi = c * IVS_PER_CORE + iv
            fbase = gi * INTERVAL
            i_tok = src[fbase + I_POS]  # [PATCHES, HIDDEN]
            # runt patches [192:196): SAD on host (f64; margin >> f32 noise)
            runt = src[fbase : fbase + INTERVAL, RUNT_START:PATCHES, :]
            sad_r = np.abs(
                runt.astype(np.float64)
                - i_tok[RUNT_START:PATCHES][None].astype(np.float64)
            ).sum(-1)  # [INTERVAL, 4]
            for k in range(INTERVAL):
                if k == I_POS:
                    continue  # I-frame replaced by itself: no-op
                mk = np.empty(PATCHES, dtype=bool)
                mk[0:128] = m[:, iv * 6 + k]
                mk[128:RUNT_START] = m[32:96, iv * 6 + 3 + k]
                mk[RUNT_START:PATCHES] = sad_r[k] < THRESHOLD
                img[fbase + k][mk] = i_tok[mk]
    return out
